# revision 14
# baseline (speedup 1.0000x reference)
"""PointNetFeaturePropagation Trainium2 kernel (8-core SPMD, data-parallel over batch).

Pipeline per core (2 batches):
  - dists: PE fp32 matmul 2<x1,x2> (bitwise-matches XLA einsum) + fused DVE
    affine_then_add assembly -> M = -(dists) bitwise-matching the reference.
  - top-3: DVE InstMax (top-8 values) + InstMaxIndex (jax top_k tie semantics).
  - interp weights: reciprocal + normalize on DVE.
  - feature gather: SWDGE dma_gather rows of p2^T, then weight-scale (DVE) +
    PE transpose to [d, n] layout.
  - MLP1 (512->256) fp32 PE matmul, training-mode BN stats via ACT accum +
    cross-core AllReduce, fused scale/shift ReLU on ACT.
  - MLP2 (256->128), second BN AllReduce, fused ReLU, DMA out.
"""

import numpy as np
from contextlib import ExitStack

B_FULL, N, S, D = 16, 4096, 1024, 128
NCORES = 8
BPC = B_FULL // NCORES  # batches per core
NT = N // 128           # 32 n-tiles per batch
NGRP = 8                # groups of 4 n-tiles (512 queries)
GT = NT // NGRP         # 4 tiles per group
EPS_INTERP = 1e-8
EPS_BN = 1e-5

_CACHED = {}


def _build_kernel():
    import concourse.bacc as bacc
    import concourse.mybir as mybir
    from concourse.tile import TileContext
    from concourse.alu_op_type import AluOpType

    f32 = mybir.dt.float32
    u16 = mybir.dt.uint16
    i16 = mybir.dt.int16
    AX = mybir.AxisListType.X
    ACTF = mybir.ActivationFunctionType

    nc = bacc.Bacc(None, target_bir_lowering=False, debug=False)

    def din(name, shape, dt=f32):
        return nc.dram_tensor(name, shape, dt, kind="ExternalInput")

    i_x1s2 = din("x1s2", [BPC, NT, 3, 128])
    i_x2r = din("x2r", [BPC, 3, S])
    i_brep = din("brep", [BPC, 128, S])
    i_negA = din("negA", [BPC, 128, NT])
    i_pts1 = din("pts1", [BPC, 128, N])
    i_p2t = din("p2t", [BPC, S, D])
    i_w0 = din("w0th", [128, 4, 2, 128])
    i_w1 = din("w1th", [128, 2, 128])
    i_g0 = din("g0", [128, 2])
    i_be0 = din("be0", [128, 2])
    i_g1 = din("g1", [128, 1])
    i_be1 = din("be1", [128, 1])
    i_eye = din("eye", [128, 128])
    o_out = nc.dram_tensor("out", [BPC, 128, N], f32, kind="ExternalOutput")


    with TileContext(nc) as tc, ExitStack() as ctx:
        cp = ctx.enter_context(tc.tile_pool(name="const", bufs=1))
        mp = ctx.enter_context(tc.tile_pool(name="msb", bufs=3))
        gp = ctx.enter_context(tc.tile_pool(name="gath", bufs=2))
        wp = ctx.enter_context(tc.tile_pool(name="wg", bufs=3))
        xp = ctx.enter_context(tc.tile_pool(name="xs", bufs=2))
        sp = ctx.enter_context(tc.tile_pool(name="scr", bufs=2))
        dq = ctx.enter_context(tc.tile_pool(name="dram", bufs=1, space="DRAM"))
        pp = ctx.enter_context(tc.tile_pool(name="pp", bufs=2, space="PSUM"))
        pt = ctx.enter_context(tc.tile_pool(name="pt", bufs=2, space="PSUM"))
        py = ctx.enter_context(tc.tile_pool(name="py", bufs=2, space="PSUM"))

        # ---- constants / staging ----
        def load(name, src, shape, dt=f32):
            t = cp.tile(shape, dt, tag=name, name=name)
            nc.sync.dma_start(t[:], src)
            return t

        brep = [load(f"brep{b}", i_brep[b], [128, S]) for b in range(BPC)]
        negA = [load(f"negA{b}", i_negA[b], [128, NT]) for b in range(BPC)]
        x2r = [load(f"x2r{b}", i_x2r[b], [3, S]) for b in range(BPC)]
        w0t = load("w0th", i_w0[:], [128, 4, 2, 128])
        w1t = load("w1th", i_w1[:], [128, 2, 128])
        g0 = load("g0", i_g0[:], [128, 2])
        be0 = load("be0", i_be0[:], [128, 2])
        g1 = load("g1", i_g1[:], [128, 1])
        be1 = load("be1", i_be1[:], [128, 1])
        eye = load("eye", i_eye[:], [128, 128])

        vst = [cp.tile([128, NT, 8], f32, tag=f"vst{b}", name=f"vst{b}") for b in range(BPC)]
        ist = [cp.tile([128, 8, NT], u16, tag=f"ist{b}", name=f"ist{b}") for b in range(BPC)]
        wst = [cp.tile([128, NT, 3], f32, tag=f"wst{b}", name=f"wst{b}") for b in range(BPC)]
        wrapped16 = [cp.tile([16, 3, 256], i16, tag=f"wrap16{b}", name=f"wrap16{b}") for b in range(BPC)]
        wrapped = [cp.tile([128, 3, 256], i16, tag=f"wrap{b}", name=f"wrap{b}") for b in range(BPC)]
        y_sb = [cp.tile([128, BPC * N], f32, tag=f"y{ot}", name=f"y{ot}") for ot in range(2)]
        z_sb = cp.tile([128, BPC * N], f32, tag="z")
        accY = cp.tile([128, 2, 16], f32, tag="accY")
        accY2 = cp.tile([128, 2, 16], f32, tag="accY2")
        accZ = cp.tile([128, 16], f32, tag="accZ")
        accZ2 = cp.tile([128, 8], f32, tag="accZ2")
        stats0 = cp.tile([128, 4], f32, tag="stats0")
        stats1 = cp.tile([128, 2], f32, tag="stats1")
        bn0 = cp.tile([128, 10], f32, tag="bn0")  # mean2|ex2|var2|sd2|s0_2... packed
        bn1 = cp.tile([128, 5], f32, tag="bn1")
        idxdram = [dq.tile([3, 128, NT], i16, tag=f"idxd{b}", name=f"idxd{b}") for b in range(BPC)]
        cin0 = dq.tile([128, 4], f32, tag="cin0")
        cout0 = dq.tile([128, 4], f32, tag="cout0")
        cin1 = dq.tile([128, 2], f32, tag="cin1")
        cout1 = dq.tile([128, 2], f32, tag="cout1")
        epsb = cp.tile([128, 1], f32, tag="epsb")
        nc.vector.memset(epsb[:], float(EPS_BN))

        def dist_topk(b):
            for jn in range(NT):
                pm = pp.tile([128, S], f32, tag="pm")
                lhs = wp.tile([3, 128], f32, tag="lhs", name="lhs")
                nc.sync.dma_start(lhs[:], i_x1s2[b, jn])
                for h in range(2):
                    sl = slice(h * 512, (h + 1) * 512)
                    nc.tensor.matmul(pm[:, sl], lhs, x2r[b][:, sl], start=True, stop=True)
                msb = mp.tile([128, S], f32, tag="m")
                # M = ((brep * -1) + (-A)) + 2P  == -(dists) bitwise
                nc.vector.affine_then_add(
                    out=msb[:], in0=brep[b][:], in1=pm[:],
                    scale=-1.0, bias=negA[b][:, jn:jn + 1])
                nc.vector.max(out=vst[b][:, jn, :], in_=msb[:])
                nc.vector.max_index(ist[b][:, :, jn], vst[b][:, jn, :], msb[:])

        def weights(b):
            # d_k = -V[:, :, :3]; w = (1/(d+eps)) normalized
            dd = sp.tile([128, NT, 3], f32, tag="dd")
            nc.vector.tensor_scalar(
                dd[:], vst[b][:, :, 0:3], -1.0, float(EPS_INTERP),
                op0=AluOpType.mult, op1=AluOpType.add)
            rec = sp.tile([128, NT, 3], f32, tag="rec")
            nc.vector.reciprocal(rec[:], dd[:])
            rs = sp.tile([128, NT], f32, tag="rs")
            nc.vector.tensor_reduce(rs[:], rec[:], AX, AluOpType.add)
            rr = sp.tile([128, NT], f32, tag="rr")
            nc.vector.reciprocal(rr[:], rs[:])
            nc.vector.tensor_tensor(
                wst[b][:], rec[:], rr[:].to_broadcast([128, NT, 3]),
                op=AluOpType.mult)

        def idx_reformat(b):
            # ist [128, 8slots, 32jn] slots 0..2 -> idxdram [3, 128, 32] (k, p, jn)
            nc.sync.dma_start(idxdram[b].rearrange("k p j -> p k j"), ist[b][:, 0:3, :].bitcast(i16))
            # idxdram -> wrapped [16, 3, 256]: wrapped[p,k,j] = idx_k[j*16+p]
            # src elem addr (k-block): (p + 16*(j%8))*32 + j//8
            for k in range(3):
                for e in range(8):
                    dst_v = wrapped16[b][:, k, :].rearrange("p (a e) -> p e a", e=8)[:, e, :]
                    nc.sync.dma_start(dst_v, idxdram[b][k, e * 16:(e + 1) * 16, :])
            for g in range(8):
                nc.sync.dma_start(wrapped[b][g * 16:(g + 1) * 16, :, :], wrapped16[b][:])

        def gather_grp(b, grp):
            gts = []
            for k in range(3):
                g = gp.tile([128, GT, D], f32, tag=f"g{k}")
                nc.gpsimd.dma_gather(
                    out_ap=g[:],
                    in_ap=i_p2t[b],
                    idxs_ap=wrapped[b][:, k, grp * 32:(grp + 1) * 32],
                    num_idxs=GT * 128,
                    num_idxs_reg=GT * 128,
                    elem_size=D,
                )
                gts.append(g)
            return gts

        def interp_x(b, grp, gts):
            xk = [xp.tile([128, 512], f32, tag=f"x{k}", name=f"x{k}") for k in range(4)]
            nc.sync.dma_start(xk[0][:], i_pts1[b][:, grp * 512:(grp + 1) * 512])
            for k in range(3):
                for j in range(GT):
                    jn = grp * GT + j
                    wg = wp.tile([128, 128], f32, tag="wg")
                    nc.vector.tensor_scalar(
                        wg[:], gts[k][:, j, :], wst[b][:, jn, k:k + 1], None,
                        op0=AluOpType.mult)
                    ptt = pt.tile([128, 128], f32, tag="ptt")
                    nc.tensor.matmul(ptt[:], wg[:], eye[:], start=True, stop=True)
                    nc.scalar.copy(xk[k + 1][:, j * 128:(j + 1) * 128], ptt[:])
            return xk

        def mlp1(b, grp, xk):
            gidx = b * NGRP + grp
            col = gidx * 512
            for ot in range(2):
                pyt = py.tile([128, 512], f32, tag="py")
                for kt in range(4):
                    nc.tensor.matmul(
                        pyt[:], w0t[:, kt, ot, :], xk[kt][:],
                        start=(kt == 0), stop=(kt == 3))
                nc.scalar.activation(
                    y_sb[ot][:, col:col + 512], pyt[:], ACTF.Copy,
                    accum_out=accY[:, ot, gidx:gidx + 1])

        # ================= phase 1: per batch =================
        for b in range(BPC):
            dist_topk(b)
            weights(b)
            idx_reformat(b)
            for grp in range(NGRP):
                gts = gather_grp(b, grp)
                xk = interp_x(b, grp, gts)
                mlp1(b, grp, xk)

        # ================= BN0 =================
        for ot in range(2):
            for i in range(8):
                scr = sp.tile([128, 1024], f32, tag="sq")
                nc.scalar.activation(
                    scr[:], y_sb[ot][:, i * 1024:(i + 1) * 1024], ACTF.Square,
                    accum_out=accY2[:, ot, i:i + 1])
        nc.vector.tensor_reduce(stats0[:, 0:1], accY[:, 0, :], AX, AluOpType.add)
        nc.vector.tensor_reduce(stats0[:, 1:2], accY2[:, 0, 0:8], AX, AluOpType.add)
        nc.vector.tensor_reduce(stats0[:, 2:3], accY[:, 1, :], AX, AluOpType.add)
        nc.vector.tensor_reduce(stats0[:, 3:4], accY2[:, 1, 0:8], AX, AluOpType.add)
        nc.sync.dma_start(cin0[:], stats0[:])
        nc.gpsimd.collective_compute(
            "AllReduce", AluOpType.add, replica_groups=[list(range(NCORES))],
            ins=[cin0[:]], outs=[cout0[:]])
        allst = sp.tile([128, 4], f32, tag="allst")
        nc.sync.dma_start(allst[:], cout0[:])
        inv_n = 1.0 / (B_FULL * N)
        # mean/ex2 for both ot: bn0 cols: 0:2 mean, 2:4 ex2, 4:6 var, 6:8 rsd, 8:10 t0
        nc.vector.tensor_scalar(bn0[:, 0:2], allst[:].rearrange("p (a two) -> p a two", two=2)[:, :, 0], inv_n, None, op0=AluOpType.mult)
        nc.vector.tensor_scalar(bn0[:, 2:4], allst[:].rearrange("p (a two) -> p a two", two=2)[:, :, 1], inv_n, None, op0=AluOpType.mult)
        m2 = sp.tile([128, 2], f32, tag="m2")
        nc.vector.tensor_tensor(m2[:], bn0[:, 0:2], bn0[:, 0:2], op=AluOpType.mult)
        nc.vector.tensor_tensor(bn0[:, 4:6], bn0[:, 2:4], m2[:], op=AluOpType.subtract)
        sd = sp.tile([128, 2], f32, tag="sd")
        nc.scalar.activation(sd[:], bn0[:, 4:6], ACTF.Sqrt, bias=epsb[:], scale=1.0)
        rsd = sp.tile([128, 2], f32, tag="rsd")
        nc.vector.reciprocal(rsd[:], sd[:])
        s0 = sp.tile([128, 2], f32, tag="s0")
        nc.vector.tensor_tensor(s0[:], g0[:], rsd[:], op=AluOpType.mult)
        ms = sp.tile([128, 2], f32, tag="ms")
        nc.vector.tensor_tensor(ms[:], bn0[:, 0:2], s0[:], op=AluOpType.mult)
        t0 = sp.tile([128, 2], f32, tag="t0")
        nc.vector.tensor_tensor(t0[:], be0[:], ms[:], op=AluOpType.subtract)

        # ================= h = relu(y*s0 + t0) in place =================
        for ot in range(2):
            for i in range(2):
                nc.scalar.activation(
                    y_sb[ot][:, i * 4096:(i + 1) * 4096],
                    y_sb[ot][:, i * 4096:(i + 1) * 4096],
                    ACTF.Relu, bias=t0[:, ot:ot + 1], scale=s0[:, ot:ot + 1])

        # ================= MLP2 =================
        for ch in range(16):
            col = ch * 512
            pz = py.tile([128, 512], f32, tag="py")
            for kt in range(2):
                nc.tensor.matmul(
                    pz[:], w1t[:, kt, :], y_sb[kt][:, col:col + 512],
                    start=(kt == 0), stop=(kt == 1))
            nc.scalar.activation(
                z_sb[:, col:col + 512], pz[:], ACTF.Copy,
                accum_out=accZ[:, ch:ch + 1])
        for i in range(8):
            scr = sp.tile([128, 1024], f32, tag="sq")
            nc.scalar.activation(
                scr[:], z_sb[:, i * 1024:(i + 1) * 1024], ACTF.Square,
                accum_out=accZ2[:, i:i + 1])
        nc.vector.tensor_reduce(stats1[:, 0:1], accZ[:], AX, AluOpType.add)
        nc.vector.tensor_reduce(stats1[:, 1:2], accZ2[:], AX, AluOpType.add)
        nc.sync.dma_start(cin1[:], stats1[:])
        nc.gpsimd.collective_compute(
            "AllReduce", AluOpType.add, replica_groups=[list(range(NCORES))],
            ins=[cin1[:]], outs=[cout1[:]])
        allst1 = sp.tile([128, 2], f32, tag="allst1")
        nc.sync.dma_start(allst1[:], cout1[:])
        nc.vector.tensor_scalar(bn1[:, 0:2], allst1[:], inv_n, None, op0=AluOpType.mult)
        m21 = sp.tile([128, 1], f32, tag="m21")
        nc.vector.tensor_tensor(m21[:], bn1[:, 0:1], bn1[:, 0:1], op=AluOpType.mult)
        nc.vector.tensor_tensor(bn1[:, 2:3], bn1[:, 1:2], m21[:], op=AluOpType.subtract)
        sd1 = sp.tile([128, 1], f32, tag="sd1")
        nc.scalar.activation(sd1[:], bn1[:, 2:3], ACTF.Sqrt, bias=epsb[:], scale=1.0)
        rsd1 = sp.tile([128, 1], f32, tag="rsd1")
        nc.vector.reciprocal(rsd1[:], sd1[:])
        s1 = sp.tile([128, 1], f32, tag="s1")
        nc.vector.tensor_tensor(s1[:], g1[:], rsd1[:], op=AluOpType.mult)
        ms1 = sp.tile([128, 1], f32, tag="ms1")
        nc.vector.tensor_tensor(ms1[:], bn1[:, 0:1], s1[:], op=AluOpType.mult)
        t1 = sp.tile([128, 1], f32, tag="t1")
        nc.vector.tensor_tensor(t1[:], be1[:], ms1[:], op=AluOpType.subtract)

        for i in range(8):
            nc.scalar.activation(
                z_sb[:, i * 1024:(i + 1) * 1024], z_sb[:, i * 1024:(i + 1) * 1024],
                ACTF.Relu, bias=t1[:], scale=s1[:])
        for b in range(BPC):
            nc.sync.dma_start(o_out[b], z_sb[:, b * N:(b + 1) * N])

    nc.finalize()
    return nc


def _prep_inputs(xyz1, xyz2, points1, points2, w0, w1, gamma0, beta0, gamma1, beta1):
    f32 = np.float32
    in_maps = []
    eye = np.eye(128, dtype=f32)
    w0th = np.ascontiguousarray(
        w0.T.reshape(4, 128, 2, 128).transpose(1, 0, 2, 3))  # [p, kt, ot, m]
    w1th = np.ascontiguousarray(w1.T.reshape(2, 128, 128).transpose(1, 0, 2))
    g0 = np.ascontiguousarray(gamma0.reshape(2, 128).T)
    be0 = np.ascontiguousarray(beta0.reshape(2, 128).T)
    g1 = gamma1.reshape(128, 1).astype(f32)
    be1 = beta1.reshape(128, 1).astype(f32)
    for c in range(NCORES):
        bs = slice(c * BPC, (c + 1) * BPC)
        x1 = xyz1[bs].astype(f32)       # [2, 3, N]
        x2 = xyz2[bs].astype(f32)       # [2, 3, S]
        # lhsT pack: [b, jn*3+c, m] = 2*x1[b, c, jn*128+m]
        x1p = (2.0 * x1).reshape(BPC, 3, NT, 128).transpose(0, 2, 1, 3)
        x2p = x2
        # sequential-order sums of squares (bitwise == XLA reduce)
        A = ((x1[:, 0] ** 2 + x1[:, 1] ** 2).astype(f32) + x1[:, 2] ** 2).astype(f32)
        Bn = ((x2[:, 0] ** 2 + x2[:, 1] ** 2).astype(f32) + x2[:, 2] ** 2).astype(f32)
        negA = (-A).reshape(BPC, NT, 128).transpose(0, 2, 1)  # [b, p, jn]
        in_maps.append({
            "x1s2": np.ascontiguousarray(x1p),
            "x2r": np.ascontiguousarray(x2p),
            "brep": np.ascontiguousarray(np.broadcast_to(Bn[:, None, :], (BPC, 128, S))),
            "negA": np.ascontiguousarray(negA),
            "pts1": np.ascontiguousarray(points1[bs].astype(f32)),
            "p2t": np.ascontiguousarray(points2[bs].transpose(0, 2, 1).astype(f32)),
            "w0th": w0th, "w1th": w1th,
            "g0": g0, "be0": be0, "g1": g1, "be1": be1,
            "eye": eye,
        })
    return in_maps


def kernel(xyz1, xyz2, points1, points2, w0, b0, gamma0, beta0, w1, b1,
           gamma1, beta1):
    # b0/b1 cancel inside training-mode BatchNorm (shift removed by mean
    # subtraction), so they are not sent to the device.
    from concourse.bass_utils import run_bass_kernel_spmd

    if "nc" not in _CACHED:
        _CACHED["nc"] = _build_kernel()
    nc = _CACHED["nc"]
    in_maps = _prep_inputs(np.asarray(xyz1), np.asarray(xyz2),
                           np.asarray(points1), np.asarray(points2),
                           np.asarray(w0), np.asarray(w1),
                           np.asarray(gamma0), np.asarray(beta0),
                           np.asarray(gamma1), np.asarray(beta1))
    res = run_bass_kernel_spmd(nc, in_maps, list(range(NCORES)))
    out = np.concatenate([res.results[i]["out"] for i in range(NCORES)], axis=0)
    return out.astype(np.float32)
